# revision 1
# baseline (speedup 1.0000x reference)
"""CBOW negative-sampling loss kernel for 8 trn2 NeuronCores.

Strategy (data-parallel over batch):
  - Host concatenates W_target/W_context into one bf16 table [2V, D] and
    builds per-batch-element combined row indices [B, 17]
    (target, context+V, neg_0+V..neg_14+V).
  - Each core handles B/8 = 16384 batch elements, 128 tiles of 128.
  - Per tile: one indirect (gather) DMA pulls 17*128 rows of 256B from HBM
    into SBUF with batch on partitions; DVE computes
    emb_in = emb_t * mask, prods = emb_in * emb_j, tree-folds the 128-d
    segments, reduces to 16 scores; ACT computes ln(sigmoid(-x)) with a
    fused per-partition accumulation (= -softplus(x) summed over j).
  - Final: per-core [128,1] f32 partial sums -> host sum -> loss.
"""

import os

import numpy as np
import ml_dtypes

import concourse.bass as bass
import concourse.mybir as mybir
import concourse.tile as tile
from concourse import bacc, bass_utils

V, D, B, NEGS = 100000, 128, 131072, 15
NCORES = 8
BLOC = B // NCORES  # 16384
P = 128
T = BLOC // P  # 128 tiles per core
J = 2 + NEGS  # 17 gathered rows per batch element
G = 4  # tiles per gather call

BF16 = mybir.dt.bfloat16
F32 = mybir.dt.float32
NPBF16 = ml_dtypes.bfloat16

_CACHE = {}
LAST_RESULT = None  # BassKernelResults of the most recent run (for profiling)


def _get_dot_scan_op():
    """Register (once) a custom DVE op: out = running-sum of Src0*Src1 over
    the streamed free dims (fp32 state, downcast out). Segment sums are then
    strided differences of the stream at segment boundaries."""
    from concourse import dve_ops as D

    if "DOT_SCAN_ANT" in D._SUB_OPCODE_FOR_NAME:
        return _CACHE["dot_scan"]
    from concourse.dve_spec import AluOp, Spec, Src0, Src1, lower, scan
    from concourse.dve_uop import DveOpSpec

    def _ref(in0, in1, *_unused):
        p = in0.shape[0]
        a = in0.astype(np.float32).reshape(p, -1)
        b = np.asarray(in1).astype(np.float32).reshape(p, -1)
        if b.shape[1] != a.shape[1]:
            reps = a.shape[1] // b.shape[1]
            b = np.tile(b.reshape(p, 1, -1), (1, reps, 1)).reshape(p, -1)
        return np.cumsum(a * b, axis=-1).astype(in0.dtype).reshape(in0.shape)

    spec = Spec(body=scan(AluOp.ADD, Src0 * Src1), reference=_ref)
    row = max(D._SUB_OPCODE_FOR_NAME.values()) + 1
    shas = {}
    for ver in ("v3", "v4"):
        s = DveOpSpec(
            name="DOT_SCAN_ANT", opcode=row, uops=lower(spec, ver=ver), rd1_en=True
        )
        shas[ver] = s.sha(ver)
    op = D.DveOp("DOT_SCAN_ANT", spec, subdim=False, uops_sha=shas)
    D.OPS.append(op)
    D._SUB_OPCODE_FOR_NAME[op.name] = row
    D.CUSTOM_DVE_SPECS[op.name] = op.spec
    _CACHE["dot_scan"] = op
    return op


def _build_nc(V=V, T=T, G=G):
    nc = bacc.Bacc("TRN2", target_bir_lowering=False, debug=False)
    w = nc.dram_tensor("w_cat", [2 * V, D], BF16, kind="ExternalInput")
    idx = nc.dram_tensor("idx", [P, T * J], mybir.dt.int32, kind="ExternalInput")
    mask = nc.dram_tensor("maskr", [P, T * D], BF16, kind="ExternalInput")
    out = nc.dram_tensor("out", [P, 1], F32, kind="ExternalOutput")

    with tile.TileContext(nc) as tc:
        with (
            tc.tile_pool(name="const", bufs=1) as constp,
            tc.tile_pool(name="gather", bufs=5) as gatherp,
            tc.tile_pool(name="work", bufs=3) as workp,
            tc.tile_pool(name="small", bufs=3) as smallp,
        ):
            idx_sb = constp.tile([P, T * J], mybir.dt.int32)
            # first group's indices land first so gather 0 launches early
            nc.sync.dma_start(idx_sb[:, 0 : G * J], idx[:, 0 : G * J])
            nc.sync.dma_start(idx_sb[:, G * J :], idx[:, G * J :])
            mask_sb = constp.tile([P, T * D], BF16)
            # chunked so the first tiles' mask arrives quickly
            MCH = 16 if T >= 16 else 1
            for m in range(MCH):
                mc = T * D // MCH
                nc.sync.dma_start(
                    mask_sb[:, m * mc : (m + 1) * mc], mask[:, m * mc : (m + 1) * mc]
                )
            # ACT function tables: sigmoid and ln live in different table
            # sets (1283ns reload on switch), so run all sigmoids in the main
            # loop and one ln+accumulate pass at the end.
            LN_CHUNKS = min(8, T // G)
            GROUPS_PER_CHUNK = (T // G) // LN_CHUNKS
            CHUNK_COLS = G * 16 * GROUPS_PER_CHUNK
            tsum = constp.tile([P, LN_CHUNKS], F32)
            ln_scratch = constp.tile([P, CHUNK_COLS], F32)

            for g in range(T // G):
                emb = gatherp.tile([P, G * J * D], BF16, tag="emb")
                nc.gpsimd.indirect_dma_start(
                    out=emb[:],
                    out_offset=None,
                    in_=w[:],
                    in_offset=bass.IndirectOffsetOnAxis(
                        ap=idx_sb[:, g * G * J : (g + 1) * G * J], axis=0
                    ),
                )
                t0 = g * G
                # per-tile batch: emb_in for all G tiles in one op
                emb_in4 = smallp.tile([P, G * D], BF16, tag="embin")
                ei4 = emb_in4[:].rearrange("p (k d) -> p k d", d=D)
                nc.vector.tensor_tensor(
                    out=ei4,
                    in0=emb[:].rearrange("p (k j d) -> p k j d", j=J, d=D)[:, :, 0, :],
                    in1=mask_sb[:, t0 * D : (t0 + G) * D].rearrange(
                        "p (k d) -> p k d", d=D
                    ),
                    op=mybir.AluOpType.mult,
                )
                # fused custom DVE op: running sum of emb_j*emb_in across the
                # whole group stream; per-(tile,j) dots are then differences
                # at the 128-element segment boundaries.
                dot_scan = _get_dot_scan_op()
                scan4 = workp.tile([P, G * 16 * D], BF16, tag="prods")
                for k in range(G):
                    base = k * J * D
                    nc.vector._custom_dve(
                        dot_scan,
                        out=scan4[:, k * 16 * D : (k + 1) * 16 * D].rearrange(
                            "p (j d) -> p j d", d=D
                        ),
                        in0=emb[:, base + D : base + J * D].rearrange(
                            "p (j d) -> p j d", d=D
                        ),
                        in1=emb_in4[:, k * D : (k + 1) * D]
                        .unsqueeze(1)
                        .broadcast_to((P, 16, D)),
                    )
                S = G * 16
                bnd = scan4[:].rearrange("p (s d) -> p s d", d=D)[:, :, D - 1 : D]
                scores = smallp.tile([P, S], F32, tag="scores")
                # x[s] = cum[s] - cum[s-1] within each tile's scan; the scan
                # state resets per call, so tile-leading segments (s % 16 == 0)
                # take the raw boundary value instead (second op overwrites).
                nc.vector.tensor_tensor(
                    out=scores[:, 1:S].unsqueeze(2),
                    in0=bnd[:, 1:S, :],
                    in1=bnd[:, 0 : S - 1, :],
                    op=mybir.AluOpType.subtract,
                )
                nc.vector.tensor_copy(
                    scores[:, 0:S:16].unsqueeze(2), bnd[:, 0:S:16, :]
                )
                if g % GROUPS_PER_CHUNK == 0:
                    sig_c = smallp.tile([P, CHUNK_COLS], F32, tag="sigc")
                gc = g % GROUPS_PER_CHUNK
                nc.scalar.activation(
                    sig_c[:, gc * G * 16 : (gc + 1) * G * 16],
                    scores[:],
                    mybir.ActivationFunctionType.Sigmoid,
                    scale=-1.0,
                )
                # ln(sigmoid(-x)) = -softplus(x); accumulate per chunk,
                # interleaved so only the last chunk sits on the tail.
                if (g + 1) % GROUPS_PER_CHUNK == 0:
                    c = (g + 1) // GROUPS_PER_CHUNK - 1
                    nc.scalar.activation(
                        ln_scratch[:],
                        sig_c[:],
                        mybir.ActivationFunctionType.Ln,
                        accum_out=tsum[:, c : c + 1],
                    )

            total = constp.tile([P, 1], F32)
            nc.vector.tensor_reduce(
                total[:], tsum[:], axis=mybir.AxisListType.X, op=mybir.AluOpType.add
            )
            nc.sync.dma_start(out[:], total[:])
    nc.compile()
    return nc


def _get_nc():
    if "nc" not in _CACHE:
        _CACHE["nc"] = _build_nc()
    return _CACHE["nc"]


def kernel(target, context, neg_idx, dropout_mask, W_target, W_context):
    global LAST_RESULT
    nc = _get_nc()

    target = np.asarray(target).astype(np.int32, copy=False)
    context = np.asarray(context).astype(np.int32, copy=False)
    neg_idx = np.asarray(neg_idx).astype(np.int32, copy=False)
    dropout_mask = np.asarray(dropout_mask, dtype=np.float32)
    W_target = np.asarray(W_target, dtype=np.float32)
    W_context = np.asarray(W_context, dtype=np.float32)

    w_cat = np.ascontiguousarray(
        np.concatenate([W_target, W_context], axis=0).astype(NPBF16)
    )
    idx_cat = np.empty((B, J), np.int32)
    idx_cat[:, 0] = target
    idx_cat[:, 1] = context + V
    idx_cat[:, 2:] = neg_idx + V
    mask_bf = dropout_mask.astype(NPBF16)

    in_maps = []
    for c in range(NCORES):
        sl = slice(c * BLOC, (c + 1) * BLOC)
        idxs = np.ascontiguousarray(
            idx_cat[sl].reshape(T, P, J).transpose(1, 0, 2).reshape(P, T * J)
        )
        maskr = np.ascontiguousarray(
            mask_bf[sl].reshape(T, P, D).transpose(1, 0, 2).reshape(P, T * D)
        )
        in_maps.append({"w_cat": w_cat, "idx": idxs, "maskr": maskr})

    trace = bool(int(os.environ.get("KERNEL_TRACE", "0")))
    res = bass_utils.run_bass_kernel_spmd(
        nc, in_maps, core_ids=list(range(NCORES)), trace=trace
    )
    LAST_RESULT = res

    tot = 0.0
    for r in res.results:
        tot += float(r["out"].astype(np.float64).sum())
    # device accumulated sum of ln(sigmoid(-x)) = -sum of softplus(x)
    loss = -tot / B
    return np.asarray(np.float32(loss))



# revision 3
# speedup vs baseline: 1.2012x; 1.2012x over previous
"""CBOW negative-sampling loss kernel for 8 trn2 NeuronCores.

Strategy (data-parallel over batch):
  - Host concatenates W_target/W_context into one bf16 table [2V, D] and
    builds per-batch-element combined row indices (target, context+V,
    neg_0+V..neg_14+V), laid out j-major per 4-tile group so the device
    can run ONE fused dot-scan per group.
  - Each core handles B/8 = 16384 batch elements, 128 tiles of 128.
  - Per 4-tile group: one indirect (gather) DMA pulls 68*128 rows of
    256B from HBM into SBUF (4 target rows/tile first, then 16 j-rows
    j-major); DVE computes emb_in = emb_t * mask, then a custom
    2-elem/cycle DVE scan (DOT_SCAN2X_ANT, hand-written 2x_1p uop
    program) computes the running sum of emb_j * emb_in over the
    j-major stream; per-(j,tile) dots are differences at the
    128-element segment boundaries; ACT applies Softplus with fused
    per-partition accumulation (single activation table, no reloads).
  - Final: per-core [128,1] f32 partial softplus sums -> host sum ->
    loss = total / B.
"""

import os

import numpy as np
import ml_dtypes

import concourse.bass as bass
import concourse.mybir as mybir
import concourse.tile as tile
from concourse import bacc, bass_utils

V, D, B, NEGS = 100000, 128, 131072, 15
NCORES = 8
BLOC = B // NCORES  # 16384
P = 128
T = BLOC // P  # 128 tiles per core
J = 2 + NEGS  # 17 gathered rows per batch element
G = 4  # tiles per gather group
NG = T // G  # 32 groups
JD = J - 1  # 16 dot rows (context + negs)

BF16 = mybir.dt.bfloat16
F32 = mybir.dt.float32
NPBF16 = ml_dtypes.bfloat16

_CACHE = {}
LAST_RESULT = None  # BassKernelResults of the most recent run (for profiling)

USE_2X = bool(int(os.environ.get("KERNEL_2X", "1")))


def _build_2x_uops():
    """Hand-written 2x_1p uop program for the dot-scan: processes element
    PAIRS (lo, hi) at 2/cycle. Mirrors the stock TENSOR_TENSOR 2x_1p
    program (slot 9 of the gen3 firmware table) for the dual-multiply
    front end, then adds the pair-sum and the running-carry blocks.

    Written stream values are carry-after-pair in BOTH the lo and hi
    output slots; only ODD stream positions (the hi slot) therefore hold
    the true inclusive scan. The kernel only reads positions 127 mod 128
    (segment boundaries), which are always odd, so this is sufficient.
    """
    from concourse.dve_uop import (
        AluInp,
        AluOp,
        DelayInp,
        InpSel,
        OutPath,
        OutSel,
        Trigger,
        UopConfig,
    )

    # --- prime uop: zero the pipeline flops of blocks 0..3 (the carry
    # lives in block 3). ZERO constants are routed down delay chains
    # 0..3, so after repeat_count=4 cycles every relevant flop is 0
    # whether the chains are registered or flow-through.
    prime = UopConfig()
    for lane in range(1, 5):
        prime.enable_input(InpSel.ZERO, lane)
    pdp = prime.datapath_config
    # chains 0..3 ingest lanes 1..4 (all ZERO) at block 0
    pdp[0].pass_through_delay(0, 1, 2, 3)
    pdp[0].enable_alu(AluOp.BYPASS, AluInp.PREV_DELAY_0)
    pdp[1].pass_through_delay(1, 2, 3)
    pdp[1].enable_alu(AluOp.BYPASS, AluInp.PREV_DELAY_1)
    pdp[2].pass_through_delay(2, 3)
    pdp[2].enable_alu(AluOp.BYPASS, AluInp.PREV_DELAY_2)
    pdp[3].pass_through_delay(3)
    pdp[3].enable_alu(AluOp.BYPASS, AluInp.PREV_DELAY_3)
    prime.repeat_count = 4
    prime.trigger = (Trigger.COUNT, Trigger.NONE, Trigger.NONE)
    prime.next_uop = (1, 0, 0)

    # --- body uop: per cycle, m0 = lo0*lo1, m1 = hi0*hi1,
    # s = m0 + m1, carry += s; write carry to both output slots.
    body = UopConfig()
    body.enable_input(InpSel.SRC_0, 0)
    body.enable_input(InpSel.SRC_1, 1)
    body.enable_input(InpSel.SRC_0_HI, 2)
    body.enable_input(InpSel.SRC_1_HI, 3)
    body.require_inp0 = 1
    body.require_inp1 = 1
    bdp = body.datapath_config
    # block0: m0 = src0_lo * src1_lo; chains 1,2 ingest the hi pair
    bdp[0].enable_alu(AluOp.MULTIPLY, AluInp.PREV_ALU_OUT, AluInp.PREV_DELAY_0)
    bdp[0].pass_through_delay(1, 2)
    # block1: m1 = src0_hi * src1_hi; chain 0 captures m0
    bdp[1].enable_alu(AluOp.MULTIPLY, AluInp.PREV_DELAY_1, AluInp.PREV_DELAY_2)
    bdp[1].enable_delay_from_src(DelayInp.PREV_ALU_OUT, 0)
    # block2: s = m0 + m1
    bdp[2].enable_alu(AluOp.ADD, AluInp.PREV_DELAY_0, AluInp.PREV_ALU_OUT)
    # block3: carry += s  (same-stage feedback, as the 1x scan does)
    bdp[3].enable_alu(AluOp.ADD, AluInp.CURR_ALU_OUT, AluInp.PREV_ALU_OUT)
    # blocks 4..7: propagate carry to the write stage
    for k in range(4, 8):
        bdp[k].pass_through_alu()
    body.enable_output(OutSel.ALU_OUT, OutPath.WR0_LO)
    body.enable_output(OutSel.ALU_OUT, OutPath.WR0_HI)
    body.trigger = (Trigger.SRC_TENSOR_DONE, Trigger.NONE, Trigger.NONE)
    body.next_uop = (0, 0, 0)

    return [prime, body]


def _get_dot_scan_op():
    """Register (once) the custom DVE dot-scan op with a 2x_1p variant:
    out = running-sum of Src0*Src1 over the streamed free dims (fp32
    carry, bf16 out). Segment sums are strided differences of the stream
    at segment boundaries (odd positions -> exact under the 2x program)."""
    from concourse import dve_ops as Dops

    name = "DOT_SCAN2X_ANT"
    if name in Dops._SUB_OPCODE_FOR_NAME:
        return _CACHE["dot_scan"]
    from concourse.dve_spec import AluOp, Spec, Src0, Src1, lower, scan
    from concourse.dve_uop import DveOpSpec

    def _ref(in0, in1, *_unused):
        p = in0.shape[0]
        a = in0.astype(np.float32).reshape(p, -1)
        b = np.asarray(in1).astype(np.float32).reshape(p, -1)
        if b.shape[1] != a.shape[1]:
            reps = a.shape[1] // b.shape[1]
            b = np.tile(b.reshape(p, 1, -1), (1, reps, 1)).reshape(p, -1)
        return np.cumsum(a * b, axis=-1).astype(in0.dtype).reshape(in0.shape)

    spec = Spec(body=scan(AluOp.ADD, Src0 * Src1), reference=_ref)
    row = max(Dops._SUB_OPCODE_FOR_NAME.values()) + 1
    uops_1x = lower(spec, ver="v3")
    opspec = DveOpSpec(
        name=name,
        opcode=row,
        uops=uops_1x,
        uops_2x=_build_2x_uops() if USE_2X else None,
        rd1_en=True,
        perf_max=1 if USE_2X else 0,
    )
    shas = {ver: opspec.sha(ver) for ver in ("v3", "v4")}
    op = Dops.DveOp(name, spec, subdim=False, uops_sha=shas)
    Dops.OPS.append(op)
    Dops._SUB_OPCODE_FOR_NAME[op.name] = row
    Dops.CUSTOM_DVE_SPECS[op.name] = op.spec
    # compile() consults this cache first, so the hand-built spec (with
    # the 2x program) is what reaches the per-NEFF DVE table writer.
    Dops._COMPILE_CACHE[(name, "v3")] = opspec
    _CACHE["dot_scan"] = op
    return op


def _build_nc():
    nc = bacc.Bacc("TRN2", target_bir_lowering=False, debug=False)
    w = nc.dram_tensor("w_cat", [2 * V, D], BF16, kind="ExternalInput")
    idx = nc.dram_tensor("idx", [P, T * J], mybir.dt.int32, kind="ExternalInput")
    mask = nc.dram_tensor("maskr", [P, T * D], BF16, kind="ExternalInput")
    out = nc.dram_tensor("out", [P, 1], F32, kind="ExternalOutput")

    dot_scan = _get_dot_scan_op()

    with tile.TileContext(nc) as tc:
        with (
            tc.tile_pool(name="const", bufs=1) as constp,
            tc.tile_pool(name="gather", bufs=5) as gatherp,
            tc.tile_pool(name="work", bufs=3) as workp,
            tc.tile_pool(name="small", bufs=3) as smallp,
        ):
            idx_sb = constp.tile([P, T * J], mybir.dt.int32)
            # first group's indices land first so gather 0 launches early
            nc.sync.dma_start(idx_sb[:, 0 : G * J], idx[:, 0 : G * J])
            nc.sync.dma_start(idx_sb[:, G * J :], idx[:, G * J :])
            mask_sb = constp.tile([P, T * D], BF16)
            # chunked so the first tiles' mask arrives quickly
            MCH = 16
            for m in range(MCH):
                mc = T * D // MCH
                nc.sync.dma_start(
                    mask_sb[:, m * mc : (m + 1) * mc], mask[:, m * mc : (m + 1) * mc]
                )
            tsum = constp.tile([P, NG], F32)

            for g in range(NG):
                emb = gatherp.tile([P, G * J * D], BF16, tag="emb")
                nc.gpsimd.indirect_dma_start(
                    out=emb[:],
                    out_offset=None,
                    in_=w[:],
                    in_offset=bass.IndirectOffsetOnAxis(
                        ap=idx_sb[:, g * G * J : (g + 1) * G * J], axis=0
                    ),
                )
                t0 = g * G
                # emb_in for the G tiles in one op: targets are the first
                # G rows of the gather
                emb_in4 = smallp.tile([P, G * D], BF16, tag="embin")
                nc.vector.tensor_tensor(
                    out=emb_in4[:].rearrange("p (k d) -> p k d", d=D),
                    in0=emb[:, 0 : G * D].rearrange("p (k d) -> p k d", d=D),
                    in1=mask_sb[:, t0 * D : (t0 + G) * D].rearrange(
                        "p (k d) -> p k d", d=D
                    ),
                    op=mybir.AluOpType.mult,
                )
                # one fused 2x dot-scan over the whole group: stream is
                # j-major [16 j, (4 tiles x 128 d)]; in1 broadcasts the
                # G*D emb_in stream across the 16 j rows.
                scan_o = workp.tile([P, JD * G * D], BF16, tag="scan")
                inst = nc.vector._custom_dve(
                    dot_scan,
                    out=scan_o[:].rearrange("p (j x) -> p j x", j=JD),
                    in0=emb[:, G * D :].rearrange("p (j x) -> p j x", j=JD),
                    in1=emb_in4[:].unsqueeze(1).broadcast_to((P, JD, G * D)),
                )
                if USE_2X:
                    inst.perf_max = 1
                # segment boundaries: s = j*G + t, boundary value at
                # d=127 of each 128-run; dots are first differences with
                # segment-leading positions (s % G == 0) taking the raw
                # boundary value.
                S = JD * G
                bnd = scan_o[:].rearrange("p (s d) -> p s d", d=D)[:, :, D - 1 : D]
                scores = smallp.tile([P, S], F32, tag="scores")
                nc.vector.tensor_tensor(
                    out=scores[:, 1:S].unsqueeze(2),
                    in0=bnd[:, 1:S, :],
                    in1=bnd[:, 0 : S - 1, :],
                    op=mybir.AluOpType.subtract,
                )
                nc.vector.tensor_copy(
                    scores[:, 0:S:G].unsqueeze(2), bnd[:, 0:S:G, :]
                )
                # loss term: softplus(x) = ln(exp(x) + 1); Exp and Ln share
                # one ACT table (natural_log_exp_and_others) so there are
                # no table reloads. Accumulate per group; the host divides
                # by B (ln(sigmoid(-x)) = -softplus(x) absorbs the sign).
                ex = smallp.tile([P, S], F32, tag="ex")
                nc.scalar.activation(
                    ex[:], scores[:], mybir.ActivationFunctionType.Exp
                )
                sp = smallp.tile([P, S], F32, tag="sp")
                nc.scalar.activation(
                    sp[:],
                    ex[:],
                    mybir.ActivationFunctionType.Ln,
                    bias=1.0,
                    accum_out=tsum[:, g : g + 1],
                )

            total = constp.tile([P, 1], F32)
            nc.vector.tensor_reduce(
                total[:], tsum[:], axis=mybir.AxisListType.X, op=mybir.AluOpType.add
            )
            nc.sync.dma_start(out[:], total[:])
    nc.compile()
    return nc


def _get_nc():
    if "nc" not in _CACHE:
        _CACHE["nc"] = _build_nc()
    return _CACHE["nc"]


def kernel(target, context, neg_idx, dropout_mask, W_target, W_context):
    global LAST_RESULT
    nc = _get_nc()

    target = np.asarray(target).astype(np.int32, copy=False)
    context = np.asarray(context).astype(np.int32, copy=False)
    neg_idx = np.asarray(neg_idx).astype(np.int32, copy=False)
    dropout_mask = np.asarray(dropout_mask, dtype=np.float32)
    W_target = np.asarray(W_target, dtype=np.float32)
    W_context = np.asarray(W_context, dtype=np.float32)

    w_cat = np.ascontiguousarray(
        np.concatenate([W_target, W_context], axis=0).astype(NPBF16)
    )
    idx_cat = np.empty((B, J), np.int32)
    idx_cat[:, 0] = target
    idx_cat[:, 1] = context + V
    idx_cat[:, 2:] = neg_idx + V
    mask_bf = dropout_mask.astype(NPBF16)

    in_maps = []
    for c in range(NCORES):
        sl = slice(c * BLOC, (c + 1) * BLOC)
        ci = idx_cat[sl].reshape(T, P, J)  # [tile, partition, j]
        # per-group j-major slot order: [G targets (t-major)] then
        # [16 j-rows x G tiles (j-major)]
        cg = ci.reshape(NG, G, P, J)
        tgt = cg[:, :, :, 0]  # [NG, G, P]
        rest = cg[:, :, :, 1:].transpose(0, 3, 1, 2)  # [NG, 16, G, P]
        slots = np.concatenate(
            [tgt, rest.reshape(NG, JD * G, P)], axis=1
        )  # [NG, G + 16*G, P]
        idxs = np.ascontiguousarray(
            slots.reshape(NG * G * J, P).T  # [P, T*J]
        )
        maskr = np.ascontiguousarray(
            mask_bf[sl].reshape(T, P, D).transpose(1, 0, 2).reshape(P, T * D)
        )
        in_maps.append({"w_cat": w_cat, "idx": idxs, "maskr": maskr})

    trace = bool(int(os.environ.get("KERNEL_TRACE", "0")))
    res = bass_utils.run_bass_kernel_spmd(
        nc, in_maps, core_ids=list(range(NCORES)), trace=trace
    )
    LAST_RESULT = res

    tot = 0.0
    for r in res.results:
        tot += float(r["out"].astype(np.float64).sum())
    # device accumulated sum of softplus(x) over all dots
    loss = tot / B
    return np.asarray(np.float32(loss))


# revision 9
# speedup vs baseline: 1.5703x; 1.3073x over previous
"""CBOW negative-sampling loss kernel for 8 trn2 NeuronCores.

Strategy (data-parallel over batch):
  - Host concatenates W_target/W_context into one bf16 table [2V, D] and
    builds per-batch-element combined row indices (target, context+V,
    neg_0+V..neg_14+V), laid out j-major per 4-tile group so the device
    can run ONE fused dot-scan per group.
  - Each core handles B/8 = 16384 batch elements, 128 tiles of 128.
  - Per 4-tile group: one indirect (gather) DMA pulls 68*128 rows of
    256B from HBM into SBUF (4 target rows/tile first, then 16 j-rows
    j-major); DVE computes emb_in = emb_t * mask, then a custom
    2-elem/cycle DVE scan (DOT_SCAN2X_ANT, hand-written 2x_1p uop
    program) computes the running sum of emb_j * emb_in over the
    j-major stream; per-(j,tile) dots are differences at the
    128-element segment boundaries; ACT applies Softplus with fused
    per-partition accumulation (single activation table, no reloads).
  - Final: per-core [128,1] f32 partial softplus sums -> host sum ->
    loss = total / B.
"""

import os

import numpy as np
import ml_dtypes

import concourse.bass as bass
import concourse.mybir as mybir
import concourse.tile as tile
from concourse import bacc, bass_utils

V, D, B, NEGS = 100000, 128, 131072, 15
NCORES = 8
BLOC = B // NCORES  # 16384
P = 128
T = BLOC // P  # 128 tiles per core
J = 2 + NEGS  # 17 gathered rows per batch element
G = 4  # tiles per gather group
NG = T // G  # 32 groups
JD = J - 1  # 16 dot rows (context + negs)

BF16 = mybir.dt.bfloat16
F32 = mybir.dt.float32
NPBF16 = ml_dtypes.bfloat16

_CACHE = {}
LAST_RESULT = None  # BassKernelResults of the most recent run (for profiling)

USE_2X = bool(int(os.environ.get("KERNEL_2X", "1")))


def _build_2x_uops():
    """Hand-written 2x_1p uop program for the dot-scan: processes element
    PAIRS (lo, hi) at 2/cycle. Mirrors the stock TENSOR_TENSOR 2x_1p
    program (slot 9 of the gen3 firmware table) for the dual-multiply
    front end, then adds the pair-sum and the running-carry blocks.

    Written stream values are carry-after-pair in BOTH the lo and hi
    output slots; only ODD stream positions (the hi slot) therefore hold
    the true inclusive scan. The kernel only reads positions 127 mod 128
    (segment boundaries), which are always odd, so this is sufficient.
    """
    from concourse.dve_uop import (
        AluInp,
        AluOp,
        DelayInp,
        InpSel,
        OutPath,
        OutSel,
        Trigger,
        UopConfig,
    )

    # --- prime uop: zero the pipeline flops of blocks 0..3 (the carry
    # lives in block 3). ZERO constants are routed down delay chains
    # 0..3, so after repeat_count=4 cycles every relevant flop is 0
    # whether the chains are registered or flow-through.
    prime = UopConfig()
    for lane in range(1, 5):
        prime.enable_input(InpSel.ZERO, lane)
    pdp = prime.datapath_config
    # chains 0..3 ingest lanes 1..4 (all ZERO) at block 0
    pdp[0].pass_through_delay(0, 1, 2, 3)
    pdp[0].enable_alu(AluOp.BYPASS, AluInp.PREV_DELAY_0)
    pdp[1].pass_through_delay(1, 2, 3)
    pdp[1].enable_alu(AluOp.BYPASS, AluInp.PREV_DELAY_1)
    pdp[2].pass_through_delay(2, 3)
    pdp[2].enable_alu(AluOp.BYPASS, AluInp.PREV_DELAY_2)
    pdp[3].pass_through_delay(3)
    pdp[3].enable_alu(AluOp.BYPASS, AluInp.PREV_DELAY_3)
    prime.repeat_count = 4
    prime.trigger = (Trigger.COUNT, Trigger.NONE, Trigger.NONE)
    prime.next_uop = (1, 0, 0)

    # --- body uop: per cycle, m0 = lo0*lo1, m1 = hi0*hi1,
    # s = m0 + m1, carry += s; write carry to both output slots.
    body = UopConfig()
    body.enable_input(InpSel.SRC_0, 0)
    body.enable_input(InpSel.SRC_1, 1)
    body.enable_input(InpSel.SRC_0_HI, 2)
    body.enable_input(InpSel.SRC_1_HI, 3)
    body.require_inp0 = 1
    body.require_inp1 = 1
    bdp = body.datapath_config
    # block0: m0 = src0_lo * src1_lo; chains 1,2 ingest the hi pair
    bdp[0].enable_alu(AluOp.MULTIPLY, AluInp.PREV_ALU_OUT, AluInp.PREV_DELAY_0)
    bdp[0].pass_through_delay(1, 2)
    # block1: m1 = src0_hi * src1_hi; chain 0 captures m0
    bdp[1].enable_alu(AluOp.MULTIPLY, AluInp.PREV_DELAY_1, AluInp.PREV_DELAY_2)
    bdp[1].enable_delay_from_src(DelayInp.PREV_ALU_OUT, 0)
    # block2: s = m0 + m1
    bdp[2].enable_alu(AluOp.ADD, AluInp.PREV_DELAY_0, AluInp.PREV_ALU_OUT)
    # block3: carry += s  (same-stage feedback, as the 1x scan does)
    bdp[3].enable_alu(AluOp.ADD, AluInp.CURR_ALU_OUT, AluInp.PREV_ALU_OUT)
    # blocks 4..7: propagate carry to the write stage
    for k in range(4, 8):
        bdp[k].pass_through_alu()
    body.enable_output(OutSel.ALU_OUT, OutPath.WR0_LO)
    body.enable_output(OutSel.ALU_OUT, OutPath.WR0_HI)
    body.trigger = (Trigger.SRC_TENSOR_DONE, Trigger.NONE, Trigger.NONE)
    body.next_uop = (0, 0, 0)

    return [prime, body]


def _get_dot_scan_op():
    """Register (once) the custom DVE dot-scan op with a 2x_1p variant:
    out = running-sum of Src0*Src1 over the streamed free dims (fp32
    carry, bf16 out). Segment sums are strided differences of the stream
    at segment boundaries (odd positions -> exact under the 2x program)."""
    from concourse import dve_ops as Dops

    name = "DOT_SCAN2X_ANT"
    if name in Dops._SUB_OPCODE_FOR_NAME:
        return _CACHE["dot_scan"]
    from concourse.dve_spec import AluOp, Spec, Src0, Src1, lower, scan
    from concourse.dve_uop import DveOpSpec

    def _ref(in0, in1, *_unused):
        p = in0.shape[0]
        a = in0.astype(np.float32).reshape(p, -1)
        b = np.asarray(in1).astype(np.float32).reshape(p, -1)
        if b.shape[1] != a.shape[1]:
            reps = a.shape[1] // b.shape[1]
            b = np.tile(b.reshape(p, 1, -1), (1, reps, 1)).reshape(p, -1)
        return np.cumsum(a * b, axis=-1).astype(in0.dtype).reshape(in0.shape)

    spec = Spec(body=scan(AluOp.ADD, Src0 * Src1), reference=_ref)
    row = max(Dops._SUB_OPCODE_FOR_NAME.values()) + 1
    uops_1x = lower(spec, ver="v3")
    opspec = DveOpSpec(
        name=name,
        opcode=row,
        uops=uops_1x,
        uops_2x=_build_2x_uops() if USE_2X else None,
        rd1_en=True,
        perf_max=1 if USE_2X else 0,
    )
    shas = {ver: opspec.sha(ver) for ver in ("v3", "v4")}
    op = Dops.DveOp(name, spec, subdim=False, uops_sha=shas)
    Dops.OPS.append(op)
    Dops._SUB_OPCODE_FOR_NAME[op.name] = row
    Dops.CUSTOM_DVE_SPECS[op.name] = op.spec
    # compile() consults this cache first, so the hand-built spec (with
    # the 2x program) is what reaches the per-NEFF DVE table writer.
    Dops._COMPILE_CACHE[(name, "v3")] = opspec
    _CACHE["dot_scan"] = op
    return op


def _build_nc():
    nc = bacc.Bacc("TRN2", target_bir_lowering=False, debug=False)
    w = nc.dram_tensor("w_cat", [2 * V, D], BF16, kind="ExternalInput")
    idx = nc.dram_tensor("idx", [P, T * J], mybir.dt.int32, kind="ExternalInput")
    mask = nc.dram_tensor("maskr", [P, T * D], BF16, kind="ExternalInput")
    out = nc.dram_tensor("out", [P, 1], F32, kind="ExternalOutput")

    dot_scan = _get_dot_scan_op()

    with tile.TileContext(nc) as tc:
        with (
            tc.tile_pool(name="const", bufs=1) as constp,
            tc.tile_pool(name="gather", bufs=5) as gatherp,
            tc.tile_pool(name="work", bufs=3) as workp,
            tc.tile_pool(name="small", bufs=3) as smallp,
        ):
            idx_sb = constp.tile([P, T * J], mybir.dt.int32)
            # first group's indices land first so gather 0 launches early
            nc.sync.dma_start(idx_sb[:, 0 : G * J], idx[:, 0 : G * J])
            nc.sync.dma_start(idx_sb[:, G * J :], idx[:, G * J :])
            mask_sb = constp.tile([P, T * D], BF16)
            # chunked so the first tiles' mask arrives quickly
            MCH = 16
            for m in range(MCH):
                mc = T * D // MCH
                nc.sync.dma_start(
                    mask_sb[:, m * mc : (m + 1) * mc], mask[:, m * mc : (m + 1) * mc]
                )
            # sigmoid(-x) buffer for the whole core; one Ln+accum pass at
            # the end turns it into sum(ln(sigmoid(-x))) = -sum softplus.
            # Keeps the ACT table loads at 2 (sigmoid, then ln once).
            sig = constp.tile([P, NG * JD * G], F32)
            total = constp.tile([P, 1], F32)

            for g in range(NG):
                emb = gatherp.tile([P, G * J * D], BF16, tag="emb")
                nc.gpsimd.indirect_dma_start(
                    out=emb[:],
                    out_offset=None,
                    in_=w[:],
                    in_offset=bass.IndirectOffsetOnAxis(
                        ap=idx_sb[:, g * G * J : (g + 1) * G * J], axis=0
                    ),
                )
                t0 = g * G
                # emb_in for the G tiles in one op: targets are the first
                # G rows of the gather
                emb_in4 = smallp.tile([P, G * D], BF16, tag="embin")
                nc.vector.tensor_tensor(
                    out=emb_in4[:].rearrange("p (k d) -> p k d", d=D),
                    in0=emb[:, 0 : G * D].rearrange("p (k d) -> p k d", d=D),
                    in1=mask_sb[:, t0 * D : (t0 + G) * D].rearrange(
                        "p (k d) -> p k d", d=D
                    ),
                    op=mybir.AluOpType.mult,
                )
                # one fused 2x dot-scan over the whole group: stream is
                # j-major [16 j, (4 tiles x 128 d)]; in1 broadcasts the
                # G*D emb_in stream across the 16 j rows.
                scan_o = workp.tile([P, JD * G * D], BF16, tag="scan")
                nc.vector._custom_dve(
                    dot_scan,
                    out=scan_o[:].rearrange("p (j x) -> p j x", j=JD),
                    in0=emb[:, G * D :].rearrange("p (j x) -> p j x", j=JD),
                    in1=emb_in4[:].unsqueeze(1).broadcast_to((P, JD, G * D)),
                )
                # segment boundaries: s = j*G + t, boundary value at
                # d=127 of each 128-run; dots are first differences with
                # segment-leading positions (s % G == 0) taking the raw
                # boundary value.
                S = JD * G
                bnd = scan_o[:].rearrange("p (s d) -> p s d", d=D)[:, :, D - 1 : D]
                scores = smallp.tile([P, S], F32, tag="scores")
                nc.vector.tensor_tensor(
                    out=scores[:, 1:S].unsqueeze(2),
                    in0=bnd[:, 1:S, :],
                    in1=bnd[:, 0 : S - 1, :],
                    op=mybir.AluOpType.subtract,
                )
                nc.vector.tensor_copy(
                    scores[:, 0:S:G].unsqueeze(2), bnd[:, 0:S:G, :]
                )
                nc.scalar.activation(
                    sig[:, g * S : (g + 1) * S],
                    scores[:],
                    mybir.ActivationFunctionType.Sigmoid,
                    scale=-1.0,
                )

            ln_scratch = constp.tile([P, NG * JD * G], F32)
            nc.scalar.activation(
                ln_scratch[:],
                sig[:],
                mybir.ActivationFunctionType.Ln,
                accum_out=total[:],
            )
            nc.sync.dma_start(out[:], total[:])

    if USE_2X:
        # The Tile context replays recorded ops, so a perf_max set on the
        # emitted wrapper is lost; set it on the final module instructions
        # (byte-36 bits 7:6) so the engine dispatches the 2x uop program.
        n2x = 0
        for f in nc.m.functions:
            for blk in f.blocks:
                for i in blk.instructions:
                    if i.__class__.__name__ == "InstCustomDveAnt":
                        i.perf_max = 1
                        n2x += 1
        assert n2x == NG, f"expected {NG} custom dve instrs, found {n2x}"
    nc.compile()
    return nc


def _get_nc():
    if "nc" not in _CACHE:
        _CACHE["nc"] = _build_nc()
    return _CACHE["nc"]


def kernel(target, context, neg_idx, dropout_mask, W_target, W_context):
    global LAST_RESULT
    nc = _get_nc()

    target = np.asarray(target).astype(np.int32, copy=False)
    context = np.asarray(context).astype(np.int32, copy=False)
    neg_idx = np.asarray(neg_idx).astype(np.int32, copy=False)
    dropout_mask = np.asarray(dropout_mask, dtype=np.float32)
    W_target = np.asarray(W_target, dtype=np.float32)
    W_context = np.asarray(W_context, dtype=np.float32)

    w_cat = np.ascontiguousarray(
        np.concatenate([W_target, W_context], axis=0).astype(NPBF16)
    )
    idx_cat = np.empty((B, J), np.int32)
    idx_cat[:, 0] = target
    idx_cat[:, 1] = context + V
    idx_cat[:, 2:] = neg_idx + V
    mask_bf = dropout_mask.astype(NPBF16)

    in_maps = []
    for c in range(NCORES):
        sl = slice(c * BLOC, (c + 1) * BLOC)
        ci = idx_cat[sl].reshape(T, P, J)  # [tile, partition, j]
        # per-group j-major slot order: [G targets (t-major)] then
        # [16 j-rows x G tiles (j-major)]
        cg = ci.reshape(NG, G, P, J)
        tgt = cg[:, :, :, 0]  # [NG, G, P]
        rest = cg[:, :, :, 1:].transpose(0, 3, 1, 2)  # [NG, 16, G, P]
        slots = np.concatenate(
            [tgt, rest.reshape(NG, JD * G, P)], axis=1
        )  # [NG, G + 16*G, P]
        idxs = np.ascontiguousarray(
            slots.reshape(NG * G * J, P).T  # [P, T*J]
        )
        maskr = np.ascontiguousarray(
            mask_bf[sl].reshape(T, P, D).transpose(1, 0, 2).reshape(P, T * D)
        )
        in_maps.append({"w_cat": w_cat, "idx": idxs, "maskr": maskr})

    trace = bool(int(os.environ.get("KERNEL_TRACE", "0")))
    res = bass_utils.run_bass_kernel_spmd(
        nc, in_maps, core_ids=list(range(NCORES)), trace=trace
    )
    LAST_RESULT = res

    tot = 0.0
    for r in res.results:
        tot += float(r["out"].astype(np.float64).sum())
    # device accumulated sum of ln(sigmoid(-x)) = -sum of softplus(x)
    loss = -tot / B
    return np.asarray(np.float32(loss))
